# revision 29
# baseline (speedup 1.0000x reference)
"""YOLOv3 detection-head decode (nn_DetectionLayer) on 8 Trainium2 NeuronCores.

Layout math (per batch image):
  in : prediction [255, 52*52]   (channel-major: ch = a*85 + attr, spatial s = gj*52+gi)
  out: pred       [8112, 85]     (row r = s*3 + a, col = attr)

The decode is fused into PE matmuls that also perform the [ch, s] -> [s, (a,attr)]
transpose. Spatial positions are 8-way interleaved: the sigmoid/exp ACT pass
writes its output permuted so SBUF free index k*338+x holds position s = 8x+k.
Then for block (g, k), matmul lhsT slices are *contiguous* (fast LDWEIGHTS),
psum partition p of block k holds s = 1024g + 8p + k, and each output-DMA
descriptor covers 24 consecutive output rows = 8160 contiguous bytes.

Per (group g, phase k), one 256-wide psum block accumulates 3 matmuls
(lhsT = float32r data, single-pass; rhs = bf16 constants, all values exactly
representable: 0/1/8):

  ps[p, 0:128]   += sigI(chan[0:128])^T  @ R1    (diag: 8,8,0,0,1,...)
  ps[p, 127:255] += sigI(chan[127:255])^T @ R2   (diag, row 0 zeroed: dup ch127)
  ps[p, 0:255]   += boxI[0:8]^T @ W8             (0/1; rows: 6 exp + 2 offset)

Anchor scales are folded into the exp as exp(tw + ln(anc_w)) via ACT bias, so
W8 stays exact. Offset rows are host-precomputed (already interleaved).
PSUM holds final output values; evict to SBUF, DMA out.

All big DMAs keep the SBUF partition count a multiple of 16 so the HWDGE
sprays descriptors across all 16 SDMA rings (a 127-row DMA serializes on one
ring — measured).
"""

import numpy as np

B = 32
C = 255
G = 52
A = 3
ATTRS = 85
S = G * G            # 2704
NCORES = 8
BPC = B // NCORES    # 4 batch images per core
IK = 8               # spatial interleave factor (consecutive s per out partition)
X = S // IK          # 338 interleave columns
GRP = 128 * IK       # 1024 spatial positions per full group
NGRP = 3             # groups per batch: 1024 + 1024 + 656
MTAIL = (S - 2 * GRP) // IK  # 82 partitions in the tail group
BLK = 256            # padded psum block width (255 used)
NPR = C + 8          # predx rows: 255 channels + 6 raw tw/th + 2 offset rows

_CACHE = {}


def _build_bass():
    import concourse.bacc as bacc
    import concourse.tile as tile
    import concourse.bass as bass
    from concourse import mybir

    f32 = mybir.dt.float32
    f32r = mybir.dt.float32r
    bf16 = mybir.dt.bfloat16
    AF = mybir.ActivationFunctionType

    nc = bacc.Bacc("TRN2", target_bir_lowering=False, debug=False)

    predx = nc.dram_tensor("predx", [BPC, NPR, S], f32, kind="ExternalInput")
    r1d = nc.dram_tensor("r1d", [128, BLK], bf16, kind="ExternalInput")
    r2d = nc.dram_tensor("r2d", [128, 128], bf16, kind="ExternalInput")
    w8d = nc.dram_tensor("w8d", [8, 12], f32r, kind="ExternalInput")
    lnad = nc.dram_tensor("lnad", [8, 1], f32, kind="ExternalInput")
    out = nc.dram_tensor("out", [BPC, S * A, ATTRS], f32, kind="ExternalOutput")

    with tile.TileContext(nc) as tc:
        with (
            tc.tile_pool(name="consts", bufs=1) as cpool,
            tc.tile_pool(name="boxp", bufs=1) as bpool,
            tc.tile_pool(name="inp", bufs=2) as ipool,
            tc.tile_pool(name="stg", bufs=2) as spool,
            tc.tile_pool(name="psp", bufs=2, space=bass.MemorySpace.PSUM) as pspool,
        ):
            r1t = cpool.tile([128, BLK], bf16)
            nc.sync.dma_start(r1t[:], r1d[:])
            r2t = cpool.tile([128, 128], bf16)
            nc.sync.dma_start(r2t[:], r2d[:])
            w8t = cpool.tile([8, 12], f32r)
            nc.sync.dma_start(w8t[:], w8d[:])
            lnat = cpool.tile([8, 1], f32)
            nc.sync.dma_start(lnat[:], lnad[:])

            # box tile per batch: rows 0..5 = exp(tw/th + ln anc), rows 6,7 =
            # [8*x_off, 8*y_off]; all rows DMA'd from host-interleaved predx,
            # then exp applied in-place to rows 0..5. All exps run before the
            # sigmoids so ACT loads each function table once.
            boxts = []
            for b in range(BPC):
                boxti = bpool.tile([8, S], f32r, tag=f"boxt{b}")
                nc.sync.dma_start(
                    boxti[0:8, :], predx[b, C : C + 8, :].bitcast(f32r)
                )
                nc.scalar.activation(
                    boxti[0:6, :], boxti[0:6, :], AF.Exp, bias=lnat[0:6, 0:1]
                )
                boxts.append(boxti)

            # PE warmup: dense dummy matmuls keep the HAM activity window busy
            # while the first batch loads, so real matmuls run at 2.4 GHz.
            wps = pspool.tile([128, IK * BLK], f32, tag="ps")
            for _ in range(20):
                nc.tensor.matmul(
                    wps[0:128, 0:BLK], r1t[:, 0:128], r1t[:],
                    start=True, stop=True,
                )

            for b in range(BPC):
                rA = ipool.tile([128, S], f32, tag="rA")
                rB = ipool.tile([128, S], f32, tag="rB")
                nc.sync.dma_start(rA[:], predx[b, 0:128, :])
                nc.sync.dma_start(rB[:], predx[b, 127:255, :])
                tAi = ipool.tile([128, S], bf16, tag="tAi")
                tBi = ipool.tile([128, S], bf16, tag="tBi")
                nc.scalar.activation(tAi[:], rA[:], AF.Sigmoid)
                nc.scalar.activation(tBi[:], rB[:], AF.Sigmoid)

                stg = spool.tile([128, NGRP * IK * 255], f32, tag="stg")

                for g in range(NGRP):
                    M = 128 if g < 2 else MTAIL
                    ps = pspool.tile([128, IK * BLK], f32, tag="ps")
                    for k in range(IK):
                        x0 = X * k + 128 * g
                        base = BLK * k
                        nc.tensor.matmul(
                            ps[:M, base : base + BLK], tAi[:, x0 : x0 + M], r1t[:],
                            start=(k % 2 == 0), stop=False,
                        )
                        nc.tensor.matmul(
                            ps[:M, base + 127 : base + 255],
                            tBi[:, x0 : x0 + M], r2t[:],
                            start=False, stop=False,
                        )
                        nc.tensor.matmul(
                            ps[:M, base + 120 : base + 132],
                            boxts[b][0:8, x0 : x0 + M], w8t[:],
                            start=False, stop=(k % 2 == 1),
                        )
                    # evict psum -> stage, undoing the column-cluster permutation
                    # (see PCOL in make_inputs): 5 affine pieces.
                    psv = ps[:M, :].rearrange("p (k c) -> p k c", k=IK)
                    stv = stg[:M, 2040 * g : 2040 * (g + 1)].rearrange(
                        "p (k c) -> p k c", c=255
                    )
                    nc.vector.tensor_copy(stv[:, :, 4:85], psv[:, :, 0:81])
                    nc.vector.tensor_copy(stv[:, :, 89:128], psv[:, :, 81:120])
                    nc.vector.tensor_copy(
                        stv.rearrange("p k (a c) -> p k a c", a=A)[:, :, :, 0:4],
                        psv[:, :, 120:132].rearrange("p k (a c) -> p k a c", a=A),
                    )
                    nc.vector.tensor_copy(stv[:, :, 128:170], psv[:, :, 132:174])
                    nc.vector.tensor_copy(stv[:, :, 174:255], psv[:, :, 174:255])

                # output DMAs: descriptor = 24 consecutive out rows = 8160 B
                main_dram = out[b, 0 : 2 * 3 * GRP, :].rearrange(
                    "(g p i a) c -> p g (i a c)", g=2, p=128, i=IK, a=A
                )
                main_sbuf = stg[:, 0 : 2 * 2040].rearrange(
                    "p (g r) -> p g r", g=2
                )
                nc.sync.dma_start(main_dram, main_sbuf)
                tail0_dram = out[b, 2 * 3 * GRP : 2 * 3 * GRP + 80 * 3 * IK, :].rearrange(
                    "(p i a) c -> p (i a c)", p=80, i=IK, a=A
                )
                nc.sync.dma_start(tail0_dram, stg[0:80, 2 * 2040 : 3 * 2040])
                tail1_dram = out[b, 2 * 3 * GRP + 80 * 3 * IK : 3 * S, :].rearrange(
                    "(p i a) c -> p (i a c)", p=2, i=IK, a=A
                )
                nc.sync.dma_start(
                    tail1_dram, stg[80:MTAIL, 2 * 2040 : 3 * 2040]
                )

    nc.compile()
    return nc


def get_nc(mm_dtype=None):
    if "nc" not in _CACHE:
        _CACHE["nc"] = _build_bass()
    return _CACHE["nc"]


def make_inputs(prediction, anchors, inp_dim, num_classes):
    """Host-side constant prep + per-core input maps."""
    import ml_dtypes

    pred = np.ascontiguousarray(np.asarray(prediction, dtype=np.float32)).reshape(
        B, C, S
    )
    anchors = np.asarray(anchors, dtype=np.float32)
    inp_dim = int(inp_dim)
    num_classes = int(num_classes)
    assert num_classes + 5 == ATTRS
    stride = float(inp_dim // G)
    bf = ml_dtypes.bfloat16

    # Column-cluster permutation: psum col for (anchor, attr). Clusters the 12
    # box-fed columns at 120..131 so the box matmul streams only N=12, while
    # R2's columns stay within psum cols 127..254.
    def pcol(a, attr):
        if attr < 4:
            return 120 + 4 * a + attr
        if a == 0:
            return attr - 4
        if a == 1:
            return 81 + (attr - 4) if attr < 43 else 132 + (attr - 43)
        return 174 + (attr - 4)

    def vch(attr):  # per-channel scale: 8,8,0,0,1,...
        return stride if attr < 2 else (0.0 if attr < 4 else 1.0)

    r1 = np.zeros((128, BLK), np.float32)
    for ch in range(128):
        a, attr = divmod(ch, 85)
        if vch(attr):
            r1[ch, pcol(a, attr)] = vch(attr)
    # R2 feeds psum cols 127..254 (rhs col c -> psum col 127+c); row 0 = dup ch127
    r2 = np.zeros((128, 128), np.float32)
    for j in range(1, 128):
        a, attr = divmod(127 + j, 85)
        if vch(attr):
            r2[j, pcol(a, attr) - 127] = vch(attr)
    # W8 feeds psum cols 120..131:
    # rows [tw0, th0, tw1, th1, tw2, th2, x8_off, y8_off]
    w8 = np.zeros((8, 12), np.float32)
    for a in range(A):
        w8[2 * a + 0, 4 * a + 2] = 1.0
        w8[2 * a + 1, 4 * a + 3] = 1.0
        w8[6, 4 * a + 0] = 1.0
        w8[7, 4 * a + 1] = 1.0

    # anchor scales folded into exp(tw + ln anc)
    with np.errstate(divide="ignore"):
        lna = np.zeros((8, 1), np.float32)
        lna[0:6, 0] = np.log(anchors.reshape(-1).astype(np.float64)).astype(np.float32)

    # 8-way interleave: linear position k*X+x holds spatial s = 8x+k.
    pos = np.arange(S)
    sI = 8 * (pos % X) + pos // X

    # predx = channels + 6 raw tw/th rows + 2 offset rows, s-axis interleaved
    predx = np.zeros((B, NPR, S), np.float32)
    predx[:, 0:C] = pred[:, :, sI]
    for a in range(A):
        predx[:, C + 2 * a + 0] = predx[:, 85 * a + 2, :]
        predx[:, C + 2 * a + 1] = predx[:, 85 * a + 3, :]
    predx[:, C + 6] = (sI % G).astype(np.float32) * stride
    predx[:, C + 7] = (sI // G).astype(np.float32) * stride

    in_maps = [
        {
            "predx": np.ascontiguousarray(predx[BPC * c : BPC * (c + 1)]),
            "r1d": r1.astype(bf),
            "r2d": r2.astype(bf),
            "w8d": w8,
            "lnad": lna,
        }
        for c in range(NCORES)
    ]
    return in_maps


def kernel(prediction, anchors, inp_dim, num_classes):
    from concourse.bass_utils import run_bass_kernel_spmd

    nc = get_nc()
    in_maps = make_inputs(prediction, anchors, inp_dim, num_classes)
    res = run_bass_kernel_spmd(nc, in_maps, core_ids=list(range(NCORES)))
    out = np.concatenate([r["out"] for r in res.results], axis=0)
    return out.reshape(B, S * A, ATTRS)


# revision 30
# speedup vs baseline: 1.0290x; 1.0290x over previous
"""YOLOv3 detection-head decode (nn_DetectionLayer) on 8 Trainium2 NeuronCores.

Layout math (per batch image):
  in : prediction [255, 52*52]   (channel-major: ch = a*85 + attr, spatial s = gj*52+gi)
  out: pred       [8112, 85]     (row r = s*3 + a, col = attr)

The decode is fused into PE matmuls that also perform the [ch, s] -> [s, (a,attr)]
transpose. Spatial positions are 8-way interleaved: the sigmoid/exp ACT pass
writes its output permuted so SBUF free index k*338+x holds position s = 8x+k.
Then for block (g, k), matmul lhsT slices are *contiguous* (fast LDWEIGHTS),
psum partition p of block k holds s = 1024g + 8p + k, and each output-DMA
descriptor covers 24 consecutive output rows = 8160 contiguous bytes.

Per (group g, phase k), one 256-wide psum block accumulates 3 matmuls
(lhsT = float32r data, single-pass; rhs = bf16 constants, all values exactly
representable: 0/1/8):

  ps[p, 0:128]   += sigI(chan[0:128])^T  @ R1    (diag: 8,8,0,0,1,...)
  ps[p, 127:255] += sigI(chan[127:255])^T @ R2   (diag, row 0 zeroed: dup ch127)
  ps[p, 0:255]   += boxI[0:8]^T @ W8             (0/1; rows: 6 exp + 2 offset)

Anchor scales are folded into the exp as exp(tw + ln(anc_w)) via ACT bias, so
W8 stays exact. Offset rows are host-precomputed (already interleaved).
PSUM holds final output values; evict to SBUF, DMA out.

All big DMAs keep the SBUF partition count a multiple of 16 so the HWDGE
sprays descriptors across all 16 SDMA rings (a 127-row DMA serializes on one
ring — measured).
"""

import numpy as np

B = 32
C = 255
G = 52
A = 3
ATTRS = 85
S = G * G            # 2704
NCORES = 8
BPC = B // NCORES    # 4 batch images per core
IK = 8               # spatial interleave factor (consecutive s per out partition)
X = S // IK          # 338 interleave columns
GRP = 128 * IK       # 1024 spatial positions per full group
NGRP = 3             # groups per batch: 1024 + 1024 + 656
MTAIL = (S - 2 * GRP) // IK  # 82 partitions in the tail group
BLK = 256            # padded psum block width (255 used)
NPR = C + 16         # predx rows: 255 channels + 6 raw tw/th + 2 offset + 8 pad
                     # (pad makes the box DMA 16 rows, so it sprays)

_CACHE = {}


def _build_bass():
    import concourse.bacc as bacc
    import concourse.tile as tile
    import concourse.bass as bass
    from concourse import mybir

    f32 = mybir.dt.float32
    f32r = mybir.dt.float32r
    bf16 = mybir.dt.bfloat16
    AF = mybir.ActivationFunctionType

    nc = bacc.Bacc("TRN2", target_bir_lowering=False, debug=False)

    predx = nc.dram_tensor("predx", [BPC, NPR, S], f32, kind="ExternalInput")
    r1d = nc.dram_tensor("r1d", [128, BLK], bf16, kind="ExternalInput")
    r2d = nc.dram_tensor("r2d", [128, 128], bf16, kind="ExternalInput")
    w8d = nc.dram_tensor("w8d", [8, 12], f32r, kind="ExternalInput")
    lnad = nc.dram_tensor("lnad", [8, 1], f32, kind="ExternalInput")
    out = nc.dram_tensor("out", [BPC, S * A, ATTRS], f32, kind="ExternalOutput")

    with tile.TileContext(nc) as tc:
        with (
            tc.tile_pool(name="consts", bufs=1) as cpool,
            tc.tile_pool(name="boxp", bufs=1) as bpool,
            tc.tile_pool(name="inp", bufs=2) as ipool,
            tc.tile_pool(name="stg", bufs=2) as spool,
            tc.tile_pool(name="psp", bufs=2, space=bass.MemorySpace.PSUM) as pspool,
        ):
            r1t = cpool.tile([128, BLK], bf16)
            nc.sync.dma_start(r1t[:], r1d[:])
            r2t = cpool.tile([128, 128], bf16)
            nc.sync.dma_start(r2t[:], r2d[:])
            w8t = cpool.tile([8, 12], f32r)
            nc.sync.dma_start(w8t[:], w8d[:])
            lnat = cpool.tile([8, 1], f32)
            nc.sync.dma_start(lnat[:], lnad[:])

            # PE warmup: dense dummy matmuls keep the HAM activity window busy
            # while the first batch loads, so real matmuls run at 2.4 GHz.
            wps = pspool.tile([128, IK * BLK], f32, tag="ps")
            for _ in range(28):
                nc.tensor.matmul(
                    wps[0:128, 0:BLK], r1t[:, 0:128], r1t[:],
                    start=True, stop=True,
                )

            # front-load all channel DMAs (bufs=3 on the raw tiles) so the
            # rings deliver batch b's data well before its matmuls.
            rAs, rBs = [], []
            for b in range(BPC):
                rA = ipool.tile([128, S], f32, tag="rA", bufs=3, name=f"rA{b}")
                rB = ipool.tile([128, S], f32, tag="rB", bufs=3, name=f"rB{b}")
                nc.sync.dma_start(rA[:], predx[b, 0:128, :])
                nc.sync.dma_start(rB[:], predx[b, 127:255, :])
                rAs.append(rA)
                rBs.append(rB)

            # box tile per batch: rows 0..5 = exp(tw/th + ln anc), rows 6,7 =
            # [8*x_off, 8*y_off]; 16 rows DMA'd (8 pad rows make it spray),
            # then exp applied in-place to rows 0..5. All exps run before the
            # sigmoids so ACT loads each function table once.
            boxts = []
            for b in range(BPC):
                boxti = bpool.tile([16, S], f32r, tag=f"boxt{b}")
                nc.sync.dma_start(
                    boxti[0:16, :], predx[b, C : C + 16, :].bitcast(f32r)
                )
                nc.scalar.activation(
                    boxti[0:6, :], boxti[0:6, :], AF.Exp, bias=lnat[0:6, 0:1]
                )
                boxts.append(boxti)

            for b in range(BPC):
                tAi = ipool.tile([128, S], bf16, tag="tAi", bufs=3, name=f"tAi{b}")
                tBi = ipool.tile([128, S], bf16, tag="tBi", bufs=3, name=f"tBi{b}")
                nc.scalar.activation(tAi[:], rAs[b][:], AF.Sigmoid)
                nc.scalar.activation(tBi[:], rBs[b][:], AF.Sigmoid)

                stg = spool.tile([128, NGRP * IK * 255], f32, tag="stg")

                for g in range(NGRP):
                    M = 128 if g < 2 else MTAIL
                    ps = pspool.tile([128, IK * BLK], f32, tag="ps")
                    for k in range(IK):
                        x0 = X * k + 128 * g
                        base = BLK * k
                        nc.tensor.matmul(
                            ps[:M, base : base + BLK], tAi[:, x0 : x0 + M], r1t[:],
                            start=(k % 2 == 0), stop=False,
                        )
                        nc.tensor.matmul(
                            ps[:M, base + 127 : base + 255],
                            tBi[:, x0 : x0 + M], r2t[:],
                            start=False, stop=False,
                        )
                        nc.tensor.matmul(
                            ps[:M, base + 120 : base + 132],
                            boxts[b][0:8, x0 : x0 + M], w8t[:],
                            start=False, stop=(k % 2 == 1),
                        )
                    # evict psum -> stage, undoing the column-cluster permutation
                    # (see PCOL in make_inputs): 5 affine pieces.
                    psv = ps[:M, :].rearrange("p (k c) -> p k c", k=IK)
                    stv = stg[:M, 2040 * g : 2040 * (g + 1)].rearrange(
                        "p (k c) -> p k c", c=255
                    )
                    nc.vector.tensor_copy(stv[:, :, 4:85], psv[:, :, 0:81])
                    nc.vector.tensor_copy(stv[:, :, 89:128], psv[:, :, 81:120])
                    nc.vector.tensor_copy(
                        stv.rearrange("p k (a c) -> p k a c", a=A)[:, :, :, 0:4],
                        psv[:, :, 120:132].rearrange("p k (a c) -> p k a c", a=A),
                    )
                    nc.vector.tensor_copy(stv[:, :, 128:170], psv[:, :, 132:174])
                    nc.vector.tensor_copy(stv[:, :, 174:255], psv[:, :, 174:255])

                # output DMAs: descriptor = 24 consecutive out rows = 8160 B
                main_dram = out[b, 0 : 2 * 3 * GRP, :].rearrange(
                    "(g p i a) c -> p g (i a c)", g=2, p=128, i=IK, a=A
                )
                main_sbuf = stg[:, 0 : 2 * 2040].rearrange(
                    "p (g r) -> p g r", g=2
                )
                nc.sync.dma_start(main_dram, main_sbuf)
                tail0_dram = out[b, 2 * 3 * GRP : 2 * 3 * GRP + 80 * 3 * IK, :].rearrange(
                    "(p i a) c -> p (i a c)", p=80, i=IK, a=A
                )
                nc.sync.dma_start(tail0_dram, stg[0:80, 2 * 2040 : 3 * 2040])
                tail1_dram = out[b, 2 * 3 * GRP + 80 * 3 * IK : 3 * S, :].rearrange(
                    "(p i a) c -> p (i a c)", p=2, i=IK, a=A
                )
                nc.sync.dma_start(
                    tail1_dram, stg[80:MTAIL, 2 * 2040 : 3 * 2040]
                )

    nc.compile()
    return nc


def get_nc(mm_dtype=None):
    if "nc" not in _CACHE:
        _CACHE["nc"] = _build_bass()
    return _CACHE["nc"]


def make_inputs(prediction, anchors, inp_dim, num_classes):
    """Host-side constant prep + per-core input maps."""
    import ml_dtypes

    pred = np.ascontiguousarray(np.asarray(prediction, dtype=np.float32)).reshape(
        B, C, S
    )
    anchors = np.asarray(anchors, dtype=np.float32)
    inp_dim = int(inp_dim)
    num_classes = int(num_classes)
    assert num_classes + 5 == ATTRS
    stride = float(inp_dim // G)
    bf = ml_dtypes.bfloat16

    # Column-cluster permutation: psum col for (anchor, attr). Clusters the 12
    # box-fed columns at 120..131 so the box matmul streams only N=12, while
    # R2's columns stay within psum cols 127..254.
    def pcol(a, attr):
        if attr < 4:
            return 120 + 4 * a + attr
        if a == 0:
            return attr - 4
        if a == 1:
            return 81 + (attr - 4) if attr < 43 else 132 + (attr - 43)
        return 174 + (attr - 4)

    def vch(attr):  # per-channel scale: 8,8,0,0,1,...
        return stride if attr < 2 else (0.0 if attr < 4 else 1.0)

    r1 = np.zeros((128, BLK), np.float32)
    for ch in range(128):
        a, attr = divmod(ch, 85)
        if vch(attr):
            r1[ch, pcol(a, attr)] = vch(attr)
    # R2 feeds psum cols 127..254 (rhs col c -> psum col 127+c); row 0 = dup ch127
    r2 = np.zeros((128, 128), np.float32)
    for j in range(1, 128):
        a, attr = divmod(127 + j, 85)
        if vch(attr):
            r2[j, pcol(a, attr) - 127] = vch(attr)
    # W8 feeds psum cols 120..131:
    # rows [tw0, th0, tw1, th1, tw2, th2, x8_off, y8_off]
    w8 = np.zeros((8, 12), np.float32)
    for a in range(A):
        w8[2 * a + 0, 4 * a + 2] = 1.0
        w8[2 * a + 1, 4 * a + 3] = 1.0
        w8[6, 4 * a + 0] = 1.0
        w8[7, 4 * a + 1] = 1.0

    # anchor scales folded into exp(tw + ln anc)
    with np.errstate(divide="ignore"):
        lna = np.zeros((8, 1), np.float32)
        lna[0:6, 0] = np.log(anchors.reshape(-1).astype(np.float64)).astype(np.float32)

    # 8-way interleave: linear position k*X+x holds spatial s = 8x+k.
    pos = np.arange(S)
    sI = 8 * (pos % X) + pos // X

    # predx = channels + 6 raw tw/th rows + 2 offset rows, s-axis interleaved
    predx = np.zeros((B, NPR, S), np.float32)
    predx[:, 0:C] = pred[:, :, sI]
    for a in range(A):
        predx[:, C + 2 * a + 0] = predx[:, 85 * a + 2, :]
        predx[:, C + 2 * a + 1] = predx[:, 85 * a + 3, :]
    predx[:, C + 6] = (sI % G).astype(np.float32) * stride
    predx[:, C + 7] = (sI // G).astype(np.float32) * stride

    in_maps = [
        {
            "predx": np.ascontiguousarray(predx[BPC * c : BPC * (c + 1)]),
            "r1d": r1.astype(bf),
            "r2d": r2.astype(bf),
            "w8d": w8,
            "lnad": lna,
        }
        for c in range(NCORES)
    ]
    return in_maps


def kernel(prediction, anchors, inp_dim, num_classes):
    from concourse.bass_utils import run_bass_kernel_spmd

    nc = get_nc()
    in_maps = make_inputs(prediction, anchors, inp_dim, num_classes)
    res = run_bass_kernel_spmd(nc, in_maps, core_ids=list(range(NCORES)))
    out = np.concatenate([r["out"] for r in res.results], axis=0)
    return out.reshape(B, S * A, ATTRS)


# revision 31
# speedup vs baseline: 1.1415x; 1.1093x over previous
"""YOLOv3 detection-head decode (nn_DetectionLayer) on 8 Trainium2 NeuronCores.

Layout math (per batch image):
  in : prediction [255, 52*52]   (channel-major: ch = a*85 + attr, spatial s = gj*52+gi)
  out: pred       [8112, 85]     (row r = s*3 + a, col = attr)

The decode is fused into PE matmuls that also perform the [ch, s] -> [s, (a,attr)]
transpose. Spatial positions are 8-way interleaved: the sigmoid/exp ACT pass
writes its output permuted so SBUF free index k*338+x holds position s = 8x+k.
Then for block (g, k), matmul lhsT slices are *contiguous* (fast LDWEIGHTS),
psum partition p of block k holds s = 1024g + 8p + k, and each output-DMA
descriptor covers 24 consecutive output rows = 8160 contiguous bytes.

Per (group g, phase k), one 256-wide psum block accumulates 3 matmuls
(lhsT = float32r data, single-pass; rhs = bf16 constants, all values exactly
representable: 0/1/8):

  ps[p, 0:128]   += sigI(chan[0:128])^T  @ R1    (diag: 8,8,0,0,1,...)
  ps[p, 127:255] += sigI(chan[127:255])^T @ R2   (diag, row 0 zeroed: dup ch127)
  ps[p, 0:255]   += boxI[0:8]^T @ W8             (0/1; rows: 6 exp + 2 offset)

Anchor scales are folded into the exp as exp(tw + ln(anc_w)) via ACT bias, so
W8 stays exact. Offset rows are host-precomputed (already interleaved).
PSUM holds final output values; evict to SBUF, DMA out.

All big DMAs keep the SBUF partition count a multiple of 16 so the HWDGE
sprays descriptors across all 16 SDMA rings (a 127-row DMA serializes on one
ring — measured).
"""

import numpy as np

B = 32
C = 255
G = 52
A = 3
ATTRS = 85
S = G * G            # 2704
NCORES = 8
BPC = B // NCORES    # 4 batch images per core
IK = 8               # spatial interleave factor (consecutive s per out partition)
X = S // IK          # 338 interleave columns
GRP = 128 * IK       # 1024 spatial positions per full group
NGRP = 3             # groups per batch: 1024 + 1024 + 656
MTAIL = (S - 2 * GRP) // IK  # 82 partitions in the tail group
BLK = 256            # padded psum block width (255 used)
NPR = C + 16         # predx rows: 255 channels + 6 raw tw/th + 2 offset + 8 pad
                     # (pad makes the box DMA 16 rows, so it sprays)

_CACHE = {}


def _build_bass():
    import concourse.bacc as bacc
    import concourse.tile as tile
    import concourse.bass as bass
    from concourse import mybir

    f32 = mybir.dt.float32
    f32r = mybir.dt.float32r
    bf16 = mybir.dt.bfloat16
    AF = mybir.ActivationFunctionType

    nc = bacc.Bacc("TRN2", target_bir_lowering=False, debug=False)

    predx = nc.dram_tensor("predx", [BPC, NPR, S], f32, kind="ExternalInput")
    r1d = nc.dram_tensor("r1d", [128, BLK], bf16, kind="ExternalInput")
    r2d = nc.dram_tensor("r2d", [128, 128], bf16, kind="ExternalInput")
    w8d = nc.dram_tensor("w8d", [8, 12], f32r, kind="ExternalInput")
    lnad = nc.dram_tensor("lnad", [8, 1], f32, kind="ExternalInput")
    out = nc.dram_tensor("out", [BPC, S * A, ATTRS], f32, kind="ExternalOutput")

    with tile.TileContext(nc) as tc:
        with (
            tc.tile_pool(name="consts", bufs=1) as cpool,
            tc.tile_pool(name="boxp", bufs=1) as bpool,
            tc.tile_pool(name="inp", bufs=2) as ipool,
            tc.tile_pool(name="stg", bufs=2) as spool,
            tc.tile_pool(name="psp", bufs=2, space=bass.MemorySpace.PSUM) as pspool,
        ):
            r1t = cpool.tile([128, BLK], bf16)
            nc.sync.dma_start(r1t[:], r1d[:])
            r2t = cpool.tile([128, 128], bf16)
            nc.sync.dma_start(r2t[:], r2d[:])
            w8t = cpool.tile([8, 12], f32r)
            nc.sync.dma_start(w8t[:], w8d[:])
            lnat = cpool.tile([8, 1], f32)
            nc.sync.dma_start(lnat[:], lnad[:])

            # box tiles first: their DMAs are 1 packet/ring and must land before
            # the channel-load flood. rows 0..5 = tw/th raw (exp'd in place),
            # rows 6,7 = [8*x_off, 8*y_off]; 16 rows so the DMA sprays.
            boxts = []
            for b in range(BPC):
                boxti = bpool.tile([16, S], f32r, tag=f"boxt{b}")
                nc.sync.dma_start(
                    boxti[0:16, :], predx[b, C : C + 16, :].bitcast(f32r)
                )
                boxts.append(boxti)

            # front-load all channel DMAs (bufs=3 on the raw tiles) so the
            # rings deliver batch b's data well before its matmuls.
            rAs, rBs = [], []
            for b in range(BPC):
                rA = ipool.tile([128, S], f32, tag="rA", bufs=3, name=f"rA{b}")
                rB = ipool.tile([128, S], f32, tag="rB", bufs=3, name=f"rB{b}")
                nc.sync.dma_start(rA[:], predx[b, 0:128, :])
                nc.sync.dma_start(rB[:], predx[b, 127:255, :])
                rAs.append(rA)
                rBs.append(rB)

            def exp_box(b):
                nc.scalar.activation(
                    boxts[b][0:6, :], boxts[b][0:6, :],
                    AF.Exp, bias=lnat[0:6, 0:1],
                )

            # ACT order: exp(b0), sigmoids(b0) — unblocks batch 0 ASAP — then
            # the remaining exps, then the other sigmoids inside the loop.
            exp_box(0)
            tAis, tBis = {}, {}

            def sigmoids(b):
                tAi = ipool.tile([128, S], bf16, tag="tAi", bufs=3, name=f"tAi{b}")
                tBi = ipool.tile([128, S], bf16, tag="tBi", bufs=3, name=f"tBi{b}")
                nc.scalar.activation(tAi[:], rAs[b][:], AF.Sigmoid)
                nc.scalar.activation(tBi[:], rBs[b][:], AF.Sigmoid)
                tAis[b], tBis[b] = tAi, tBi

            sigmoids(0)
            for b in range(1, BPC):
                exp_box(b)

            for b in range(BPC):
                if b > 0:
                    sigmoids(b)
                tAi, tBi = tAis[b], tBis[b]

                stg = spool.tile([128, NGRP * IK * 255], f32, tag="stg")

                for g in range(NGRP):
                    M = 128 if g < 2 else MTAIL
                    ps = pspool.tile([128, IK * BLK], f32, tag="ps")
                    if b == 0 and g == 0:
                        # PE warmup: dense dummy matmuls into this same psum
                        # tile keep the HAM activity window busy while batch 0
                        # loads, so the real matmuls run at 2.4 GHz.
                        for _ in range(28):
                            nc.tensor.matmul(
                                ps[0:128, 0:BLK], r1t[:, 0:128], r1t[:],
                                start=True, stop=True,
                            )
                    for k in range(IK):
                        x0 = X * k + 128 * g
                        base = BLK * k
                        nc.tensor.matmul(
                            ps[:M, base : base + BLK], tAi[:, x0 : x0 + M], r1t[:],
                            start=(k % 2 == 0), stop=False,
                        )
                        nc.tensor.matmul(
                            ps[:M, base + 127 : base + 255],
                            tBi[:, x0 : x0 + M], r2t[:],
                            start=False, stop=False,
                        )
                        nc.tensor.matmul(
                            ps[:M, base + 120 : base + 132],
                            boxts[b][0:8, x0 : x0 + M], w8t[:],
                            start=False, stop=(k % 2 == 1),
                        )
                    # evict psum -> stage, undoing the column-cluster permutation
                    # (see PCOL in make_inputs): 5 affine pieces.
                    psv = ps[:M, :].rearrange("p (k c) -> p k c", k=IK)
                    stv = stg[:M, 2040 * g : 2040 * (g + 1)].rearrange(
                        "p (k c) -> p k c", c=255
                    )
                    nc.vector.tensor_copy(stv[:, :, 4:85], psv[:, :, 0:81])
                    nc.vector.tensor_copy(stv[:, :, 89:128], psv[:, :, 81:120])
                    nc.vector.tensor_copy(
                        stv.rearrange("p k (a c) -> p k a c", a=A)[:, :, :, 0:4],
                        psv[:, :, 120:132].rearrange("p k (a c) -> p k a c", a=A),
                    )
                    nc.vector.tensor_copy(stv[:, :, 128:170], psv[:, :, 132:174])
                    nc.vector.tensor_copy(stv[:, :, 174:255], psv[:, :, 174:255])

                # output DMAs: descriptor = 24 consecutive out rows = 8160 B
                main_dram = out[b, 0 : 2 * 3 * GRP, :].rearrange(
                    "(g p i a) c -> p g (i a c)", g=2, p=128, i=IK, a=A
                )
                main_sbuf = stg[:, 0 : 2 * 2040].rearrange(
                    "p (g r) -> p g r", g=2
                )
                nc.sync.dma_start(main_dram, main_sbuf)
                tail0_dram = out[b, 2 * 3 * GRP : 2 * 3 * GRP + 80 * 3 * IK, :].rearrange(
                    "(p i a) c -> p (i a c)", p=80, i=IK, a=A
                )
                nc.sync.dma_start(tail0_dram, stg[0:80, 2 * 2040 : 3 * 2040])
                tail1_dram = out[b, 2 * 3 * GRP + 80 * 3 * IK : 3 * S, :].rearrange(
                    "(p i a) c -> p (i a c)", p=2, i=IK, a=A
                )
                nc.sync.dma_start(
                    tail1_dram, stg[80:MTAIL, 2 * 2040 : 3 * 2040]
                )

    nc.compile()
    return nc


def get_nc(mm_dtype=None):
    if "nc" not in _CACHE:
        _CACHE["nc"] = _build_bass()
    return _CACHE["nc"]


def make_inputs(prediction, anchors, inp_dim, num_classes):
    """Host-side constant prep + per-core input maps."""
    import ml_dtypes

    pred = np.ascontiguousarray(np.asarray(prediction, dtype=np.float32)).reshape(
        B, C, S
    )
    anchors = np.asarray(anchors, dtype=np.float32)
    inp_dim = int(inp_dim)
    num_classes = int(num_classes)
    assert num_classes + 5 == ATTRS
    stride = float(inp_dim // G)
    bf = ml_dtypes.bfloat16

    # Column-cluster permutation: psum col for (anchor, attr). Clusters the 12
    # box-fed columns at 120..131 so the box matmul streams only N=12, while
    # R2's columns stay within psum cols 127..254.
    def pcol(a, attr):
        if attr < 4:
            return 120 + 4 * a + attr
        if a == 0:
            return attr - 4
        if a == 1:
            return 81 + (attr - 4) if attr < 43 else 132 + (attr - 43)
        return 174 + (attr - 4)

    def vch(attr):  # per-channel scale: 8,8,0,0,1,...
        return stride if attr < 2 else (0.0 if attr < 4 else 1.0)

    r1 = np.zeros((128, BLK), np.float32)
    for ch in range(128):
        a, attr = divmod(ch, 85)
        if vch(attr):
            r1[ch, pcol(a, attr)] = vch(attr)
    # R2 feeds psum cols 127..254 (rhs col c -> psum col 127+c); row 0 = dup ch127
    r2 = np.zeros((128, 128), np.float32)
    for j in range(1, 128):
        a, attr = divmod(127 + j, 85)
        if vch(attr):
            r2[j, pcol(a, attr) - 127] = vch(attr)
    # W8 feeds psum cols 120..131:
    # rows [tw0, th0, tw1, th1, tw2, th2, x8_off, y8_off]
    w8 = np.zeros((8, 12), np.float32)
    for a in range(A):
        w8[2 * a + 0, 4 * a + 2] = 1.0
        w8[2 * a + 1, 4 * a + 3] = 1.0
        w8[6, 4 * a + 0] = 1.0
        w8[7, 4 * a + 1] = 1.0

    # anchor scales folded into exp(tw + ln anc)
    with np.errstate(divide="ignore"):
        lna = np.zeros((8, 1), np.float32)
        lna[0:6, 0] = np.log(anchors.reshape(-1).astype(np.float64)).astype(np.float32)

    # 8-way interleave: linear position k*X+x holds spatial s = 8x+k.
    pos = np.arange(S)
    sI = 8 * (pos % X) + pos // X

    # predx = channels + 6 raw tw/th rows + 2 offset rows, s-axis interleaved
    predx = np.zeros((B, NPR, S), np.float32)
    predx[:, 0:C] = pred[:, :, sI]
    for a in range(A):
        predx[:, C + 2 * a + 0] = predx[:, 85 * a + 2, :]
        predx[:, C + 2 * a + 1] = predx[:, 85 * a + 3, :]
    predx[:, C + 6] = (sI % G).astype(np.float32) * stride
    predx[:, C + 7] = (sI // G).astype(np.float32) * stride

    in_maps = [
        {
            "predx": np.ascontiguousarray(predx[BPC * c : BPC * (c + 1)]),
            "r1d": r1.astype(bf),
            "r2d": r2.astype(bf),
            "w8d": w8,
            "lnad": lna,
        }
        for c in range(NCORES)
    ]
    return in_maps


def kernel(prediction, anchors, inp_dim, num_classes):
    from concourse.bass_utils import run_bass_kernel_spmd

    nc = get_nc()
    in_maps = make_inputs(prediction, anchors, inp_dim, num_classes)
    res = run_bass_kernel_spmd(nc, in_maps, core_ids=list(range(NCORES)))
    out = np.concatenate([r["out"] for r in res.results], axis=0)
    return out.reshape(B, S * A, ATTRS)


# revision 32
# speedup vs baseline: 1.1443x; 1.0025x over previous
"""YOLOv3 detection-head decode (nn_DetectionLayer) on 8 Trainium2 NeuronCores.

Layout math (per batch image):
  in : prediction [255, 52*52]   (channel-major: ch = a*85 + attr, spatial s = gj*52+gi)
  out: pred       [8112, 85]     (row r = s*3 + a, col = attr)

The decode is fused into PE matmuls that also perform the [ch, s] -> [s, (a,attr)]
transpose. Spatial positions are 8-way interleaved: the sigmoid/exp ACT pass
writes its output permuted so SBUF free index k*338+x holds position s = 8x+k.
Then for block (g, k), matmul lhsT slices are *contiguous* (fast LDWEIGHTS),
psum partition p of block k holds s = 1024g + 8p + k, and each output-DMA
descriptor covers 24 consecutive output rows = 8160 contiguous bytes.

Per (group g, phase k), one 256-wide psum block accumulates 3 matmuls
(lhsT = float32r data, single-pass; rhs = bf16 constants, all values exactly
representable: 0/1/8):

  ps[p, 0:128]   += sigI(chan[0:128])^T  @ R1    (diag: 8,8,0,0,1,...)
  ps[p, 127:255] += sigI(chan[127:255])^T @ R2   (diag, row 0 zeroed: dup ch127)
  ps[p, 0:255]   += boxI[0:8]^T @ W8             (0/1; rows: 6 exp + 2 offset)

Anchor scales are folded into the exp as exp(tw + ln(anc_w)) via ACT bias, so
W8 stays exact. Offset rows are host-precomputed (already interleaved).
PSUM holds final output values; evict to SBUF, DMA out.

All big DMAs keep the SBUF partition count a multiple of 16 so the HWDGE
sprays descriptors across all 16 SDMA rings (a 127-row DMA serializes on one
ring — measured).
"""

import numpy as np

B = 32
C = 255
G = 52
A = 3
ATTRS = 85
S = G * G            # 2704
NCORES = 8
BPC = B // NCORES    # 4 batch images per core
IK = 8               # spatial interleave factor (consecutive s per out partition)
X = S // IK          # 338 interleave columns
GRP = 128 * IK       # 1024 spatial positions per full group
NGRP = 3             # groups per batch: 1024 + 1024 + 656
MTAIL = (S - 2 * GRP) // IK  # 82 partitions in the tail group
BLK = 256            # padded psum block width (255 used)
NPR = C + 16         # predx rows: 255 channels + 6 raw tw/th + 2 offset + 8 pad
                     # (pad makes the box DMA 16 rows, so it sprays)

_CACHE = {}


def _build_bass():
    import concourse.bacc as bacc
    import concourse.tile as tile
    import concourse.bass as bass
    from concourse import mybir

    f32 = mybir.dt.float32
    f32r = mybir.dt.float32r
    bf16 = mybir.dt.bfloat16
    AF = mybir.ActivationFunctionType

    nc = bacc.Bacc("TRN2", target_bir_lowering=False, debug=False)

    predx = nc.dram_tensor("predx", [BPC, NPR, S], f32, kind="ExternalInput")
    r1d = nc.dram_tensor("r1d", [128, BLK], bf16, kind="ExternalInput")
    r2d = nc.dram_tensor("r2d", [128, 128], bf16, kind="ExternalInput")
    w8d = nc.dram_tensor("w8d", [8, 12], f32r, kind="ExternalInput")
    lnad = nc.dram_tensor("lnad", [8, 1], f32, kind="ExternalInput")
    out = nc.dram_tensor("out", [BPC, S * A, ATTRS], f32, kind="ExternalOutput")

    with tile.TileContext(nc) as tc:
        with (
            tc.tile_pool(name="consts", bufs=1) as cpool,
            tc.tile_pool(name="boxp", bufs=1) as bpool,
            tc.tile_pool(name="inp", bufs=2) as ipool,
            tc.tile_pool(name="stg", bufs=2) as spool,
            tc.tile_pool(name="psp", bufs=2, space=bass.MemorySpace.PSUM) as pspool,
        ):
            r1t = cpool.tile([128, BLK], bf16)
            nc.sync.dma_start(r1t[:], r1d[:])
            r2t = cpool.tile([128, 128], bf16)
            nc.sync.dma_start(r2t[:], r2d[:])
            w8t = cpool.tile([8, 12], f32r)
            nc.sync.dma_start(w8t[:], w8d[:])
            lnat = cpool.tile([8, 1], f32)
            nc.sync.dma_start(lnat[:], lnad[:])

            # box tiles first: their DMAs are 1 packet/ring and must land before
            # the channel-load flood. rows 0..5 = tw/th raw (exp'd in place),
            # rows 6,7 = [8*x_off, 8*y_off]; 16 rows so the DMA sprays.
            boxts = []
            for b in range(BPC):
                boxti = bpool.tile([16, S], f32r, tag=f"boxt{b}")
                nc.sync.dma_start(
                    boxti[0:16, :], predx[b, C : C + 16, :].bitcast(f32r)
                )
                boxts.append(boxti)

            # front-load all channel DMAs (bufs=3 on the raw tiles) so the
            # rings deliver batch b's data well before its matmuls.
            rAs, rBs = [], []
            for b in range(BPC):
                rA = ipool.tile([128, S], f32, tag="rA", bufs=4, name=f"rA{b}")
                rB = ipool.tile([128, S], f32, tag="rB", bufs=4, name=f"rB{b}")
                nc.sync.dma_start(rA[:], predx[b, 0:128, :])
                nc.sync.dma_start(rB[:], predx[b, 127:255, :])
                rAs.append(rA)
                rBs.append(rB)

            def exp_box(b):
                nc.scalar.activation(
                    boxts[b][0:6, :], boxts[b][0:6, :],
                    AF.Exp, bias=lnat[0:6, 0:1],
                )

            # ACT order: exp(b0), sigmoids(b0) — unblocks batch 0 ASAP — then
            # the remaining exps, then the other sigmoids inside the loop.
            exp_box(0)
            tAis, tBis = {}, {}

            def sigmoids(b):
                tAi = ipool.tile([128, S], bf16, tag="tAi", bufs=2, name=f"tAi{b}")
                tBi = ipool.tile([128, S], bf16, tag="tBi", bufs=2, name=f"tBi{b}")
                nc.scalar.activation(tAi[:], rAs[b][:], AF.Sigmoid)
                nc.scalar.activation(tBi[:], rBs[b][:], AF.Sigmoid)
                tAis[b], tBis[b] = tAi, tBi

            sigmoids(0)
            for b in range(1, BPC):
                exp_box(b)

            for b in range(BPC):
                if b > 0:
                    sigmoids(b)
                tAi, tBi = tAis[b], tBis[b]

                stg = spool.tile([128, NGRP * IK * 255], f32, tag="stg")

                for g in range(NGRP):
                    M = 128 if g < 2 else MTAIL
                    ps = pspool.tile([128, IK * BLK], f32, tag="ps")
                    if b == 0 and g == 0:
                        # PE warmup: dense dummy matmuls into this same psum
                        # tile keep the HAM activity window busy while batch 0
                        # loads, so the real matmuls run at 2.4 GHz.
                        for _ in range(60):
                            nc.tensor.matmul(
                                ps[0:128, 0:BLK], r1t[:, 0:128], r1t[:],
                                start=True, stop=True,
                            )
                    for k in range(IK):
                        x0 = X * k + 128 * g
                        base = BLK * k
                        nc.tensor.matmul(
                            ps[:M, base : base + BLK], tAi[:, x0 : x0 + M], r1t[:],
                            start=(k % 2 == 0), stop=False,
                        )
                        nc.tensor.matmul(
                            ps[:M, base + 127 : base + 255],
                            tBi[:, x0 : x0 + M], r2t[:],
                            start=False, stop=False,
                        )
                        nc.tensor.matmul(
                            ps[:M, base + 120 : base + 132],
                            boxts[b][0:8, x0 : x0 + M], w8t[:],
                            start=False, stop=(k % 2 == 1),
                        )
                    # evict psum -> stage, undoing the column-cluster permutation
                    # (see PCOL in make_inputs): 5 affine pieces.
                    psv = ps[:M, :].rearrange("p (k c) -> p k c", k=IK)
                    stv = stg[:M, 2040 * g : 2040 * (g + 1)].rearrange(
                        "p (k c) -> p k c", c=255
                    )
                    nc.vector.tensor_copy(stv[:, :, 4:85], psv[:, :, 0:81])
                    nc.vector.tensor_copy(stv[:, :, 89:128], psv[:, :, 81:120])
                    nc.vector.tensor_copy(
                        stv.rearrange("p k (a c) -> p k a c", a=A)[:, :, :, 0:4],
                        psv[:, :, 120:132].rearrange("p k (a c) -> p k a c", a=A),
                    )
                    nc.vector.tensor_copy(stv[:, :, 128:170], psv[:, :, 132:174])
                    nc.vector.tensor_copy(stv[:, :, 174:255], psv[:, :, 174:255])

                # output DMAs: descriptor = 24 consecutive out rows = 8160 B
                main_dram = out[b, 0 : 2 * 3 * GRP, :].rearrange(
                    "(g p i a) c -> p g (i a c)", g=2, p=128, i=IK, a=A
                )
                main_sbuf = stg[:, 0 : 2 * 2040].rearrange(
                    "p (g r) -> p g r", g=2
                )
                nc.sync.dma_start(main_dram, main_sbuf)
                tail0_dram = out[b, 2 * 3 * GRP : 2 * 3 * GRP + 80 * 3 * IK, :].rearrange(
                    "(p i a) c -> p (i a c)", p=80, i=IK, a=A
                )
                nc.sync.dma_start(tail0_dram, stg[0:80, 2 * 2040 : 3 * 2040])
                tail1_dram = out[b, 2 * 3 * GRP + 80 * 3 * IK : 3 * S, :].rearrange(
                    "(p i a) c -> p (i a c)", p=2, i=IK, a=A
                )
                nc.sync.dma_start(
                    tail1_dram, stg[80:MTAIL, 2 * 2040 : 3 * 2040]
                )

    nc.compile()
    return nc


def get_nc(mm_dtype=None):
    if "nc" not in _CACHE:
        _CACHE["nc"] = _build_bass()
    return _CACHE["nc"]


def make_inputs(prediction, anchors, inp_dim, num_classes):
    """Host-side constant prep + per-core input maps."""
    import ml_dtypes

    pred = np.ascontiguousarray(np.asarray(prediction, dtype=np.float32)).reshape(
        B, C, S
    )
    anchors = np.asarray(anchors, dtype=np.float32)
    inp_dim = int(inp_dim)
    num_classes = int(num_classes)
    assert num_classes + 5 == ATTRS
    stride = float(inp_dim // G)
    bf = ml_dtypes.bfloat16

    # Column-cluster permutation: psum col for (anchor, attr). Clusters the 12
    # box-fed columns at 120..131 so the box matmul streams only N=12, while
    # R2's columns stay within psum cols 127..254.
    def pcol(a, attr):
        if attr < 4:
            return 120 + 4 * a + attr
        if a == 0:
            return attr - 4
        if a == 1:
            return 81 + (attr - 4) if attr < 43 else 132 + (attr - 43)
        return 174 + (attr - 4)

    def vch(attr):  # per-channel scale: 8,8,0,0,1,...
        return stride if attr < 2 else (0.0 if attr < 4 else 1.0)

    r1 = np.zeros((128, BLK), np.float32)
    for ch in range(128):
        a, attr = divmod(ch, 85)
        if vch(attr):
            r1[ch, pcol(a, attr)] = vch(attr)
    # R2 feeds psum cols 127..254 (rhs col c -> psum col 127+c); row 0 = dup ch127
    r2 = np.zeros((128, 128), np.float32)
    for j in range(1, 128):
        a, attr = divmod(127 + j, 85)
        if vch(attr):
            r2[j, pcol(a, attr) - 127] = vch(attr)
    # W8 feeds psum cols 120..131:
    # rows [tw0, th0, tw1, th1, tw2, th2, x8_off, y8_off]
    w8 = np.zeros((8, 12), np.float32)
    for a in range(A):
        w8[2 * a + 0, 4 * a + 2] = 1.0
        w8[2 * a + 1, 4 * a + 3] = 1.0
        w8[6, 4 * a + 0] = 1.0
        w8[7, 4 * a + 1] = 1.0

    # anchor scales folded into exp(tw + ln anc)
    with np.errstate(divide="ignore"):
        lna = np.zeros((8, 1), np.float32)
        lna[0:6, 0] = np.log(anchors.reshape(-1).astype(np.float64)).astype(np.float32)

    # 8-way interleave: linear position k*X+x holds spatial s = 8x+k.
    pos = np.arange(S)
    sI = 8 * (pos % X) + pos // X

    # predx = channels + 6 raw tw/th rows + 2 offset rows, s-axis interleaved
    predx = np.zeros((B, NPR, S), np.float32)
    predx[:, 0:C] = pred[:, :, sI]
    for a in range(A):
        predx[:, C + 2 * a + 0] = predx[:, 85 * a + 2, :]
        predx[:, C + 2 * a + 1] = predx[:, 85 * a + 3, :]
    predx[:, C + 6] = (sI % G).astype(np.float32) * stride
    predx[:, C + 7] = (sI // G).astype(np.float32) * stride

    in_maps = [
        {
            "predx": np.ascontiguousarray(predx[BPC * c : BPC * (c + 1)]),
            "r1d": r1.astype(bf),
            "r2d": r2.astype(bf),
            "w8d": w8,
            "lnad": lna,
        }
        for c in range(NCORES)
    ]
    return in_maps


def kernel(prediction, anchors, inp_dim, num_classes):
    from concourse.bass_utils import run_bass_kernel_spmd

    nc = get_nc()
    in_maps = make_inputs(prediction, anchors, inp_dim, num_classes)
    res = run_bass_kernel_spmd(nc, in_maps, core_ids=list(range(NCORES)))
    out = np.concatenate([r["out"] for r in res.results], axis=0)
    return out.reshape(B, S * A, ATTRS)
